# revision 1
# baseline (speedup 1.0000x reference)
"""Multi-head attention Trainium2 kernel (B=2, S=4096, D=512, H=8).

Sharding: 8 cores = (batch b = c//4) x (query chunk qs = c%4 of 1024 rows).
Each core computes, for its 1024 query rows: full K/V projections for its
batch, scores/softmax/PV for all 8 heads, and the output projection.
No collectives needed.

Layout strategy (per core):
  - activations transposed on host: xT [D, S] so the model dim is the
    matmul contraction (partition) dim.
  - scores are computed transposed: scoresT [k, q] so that PV can consume
    probsT directly (contraction over k on partitions).
  - mask is passed as bf16 bits in natural [q, k] layout and transposed
    on-device via the DMA xbar into maskT [k-block 128, q 1024] tiles.
  - softmax without max-subtraction (scores bounded ~|44|, exp fits f32);
    denominator via an appended ones-column in V (row 64 of the PV psum).
  - matmul dtypes: f32r (tf32-like, 1 cyc/row) for projections + scores;
    bf16 for probs*V and the output projection.
"""

import sys
import types

import numpy as np
import ml_dtypes

import bass_rust
import concourse.bass as bass
import concourse.mybir as mybir
from concourse.bass_utils import run_bass_kernel_spmd
from concourse.tile import TileContext

f32 = mybir.dt.float32
f32r = mybir.dt.float32r
bf16 = mybir.dt.bfloat16
AF = mybir.ActivationFunctionType
MULT = mybir.AluOpType.mult

B, S, D, H, HD = 2, 4096, 512, 8, 64
QC = 1024          # query rows per core
NC = 8             # cores
NKB = S // 128     # 32 k-blocks
NDB = D // 128     # 4 d-blocks

_waitfix = [0]


def _legalize_waits(nc):
    """This walrus build accepts at most one sem-wait per instruction.
    Hoist extra waits onto same-engine NOPs inserted just before."""
    for fn in nc.m.functions:
        for bb in fn.blocks:
            out, changed = [], False
            for inst in bb.instructions:
                si = inst.sync_info
                if si is not None and len(si.on_wait) > 1:
                    waits = list(si.on_wait)
                    for w in waits[:-1]:
                        _waitfix[0] += 1
                        nop = mybir.InstNoOp(
                            name=f"I-waitfix-{_waitfix[0]}", ins=[], outs=[])
                        nop.engine = inst.engine
                        nop.sync_info = bass_rust.SyncInfo(on_wait=[w], on_update=[])
                        out.append(nop)
                    inst.sync_info = bass_rust.SyncInfo(
                        on_wait=[waits[-1]], on_update=list(si.on_update))
                    changed = True
                out.append(inst)
            if changed:
                bb.instructions = out


def _build_program(debug_taps=False):
    nc = bass.Bass(target_bir_lowering=False, debug=False)

    xqT = nc.dram_tensor("xqT", [D, QC], f32r, kind="ExternalInput")
    xkT = nc.dram_tensor("xkT", [D, S], f32r, kind="ExternalInput")
    xvT = nc.dram_tensor("xvT", [D, S], f32r, kind="ExternalInput")
    masktd = nc.dram_tensor("masktd", [S, QC], bf16, kind="ExternalInput")
    wqT = nc.dram_tensor("wqT", [D, D], f32r, kind="ExternalInput")
    wkT = nc.dram_tensor("wkT", [D, D], f32r, kind="ExternalInput")
    wvT = nc.dram_tensor("wvT", [D, D], f32r, kind="ExternalInput")
    woT = nc.dram_tensor("woT", [D, D], bf16, kind="ExternalInput")
    bq_d = nc.dram_tensor("bq_d", [128, NDB], f32, kind="ExternalInput")
    bk_d = nc.dram_tensor("bk_d", [128, NDB], f32, kind="ExternalInput")
    bv_d = nc.dram_tensor("bv_d", [1, D], f32r, kind="ExternalInput")
    bo_d = nc.dram_tensor("bo_d", [1, D], f32r, kind="ExternalInput")
    outp = nc.dram_tensor("out", [QC, D], f32, kind="ExternalOutput")
    ktdram = nc.dram_tensor("ktdram", [NDB, 128, S], f32r,
                            kind="ExternalOutput" if debug_taps else "Internal")
    if debug_taps:
        dbg_qt = nc.dram_tensor("dbg_qt", [NDB, 128, QC], f32, kind="ExternalOutput")
        dbg_va = nc.dram_tensor("dbg_va", [NKB, 128, 520], f32, kind="ExternalOutput")
        dbg_mk = nc.dram_tensor("dbg_mk", [NKB, 128, QC], f32, kind="ExternalOutput")
        dbg_pr = nc.dram_tensor("dbg_pr", [2, 128, QC], f32, kind="ExternalOutput")
        dbg_at = nc.dram_tensor("dbg_at", [H, 64, QC], f32, kind="ExternalOutput")
        dbg_rd = nc.dram_tensor("dbg_rd", [H, 1, QC], f32, kind="ExternalOutput")

    with TileContext(nc) as tc:
        with tc.tile_pool(name="cpool", bufs=1) as cpool, \
             tc.tile_pool(name="psAB", bufs=1, space="PSUM") as psB:
            # ---- constants ----
            ones_f = cpool.tile([1, 128], f32, tag="ones_f")
            nc.vector.memset(ones_f[:, :], 1.0)
            ones_r = cpool.tile([1, 128], f32r, tag="ones_r")
            nc.vector.tensor_copy(ones_r[:, :], ones_f[:, :])
            bq_t = cpool.tile([128, NDB], f32, tag="bq")
            bk_t = cpool.tile([128, NDB], f32, tag="bk")
            bv_t = cpool.tile([1, D], f32r, tag="bv")
            bo_t = cpool.tile([1, D], f32r, tag="bo")
            nc.scalar.dma_start(out=bq_t[:, :], in_=bq_d[:, :])
            nc.scalar.dma_start(out=bk_t[:, :], in_=bk_d[:, :])
            nc.scalar.dma_start(out=bv_t[:, :], in_=bv_d[:, :])
            nc.scalar.dma_start(out=bo_t[:, :], in_=bo_d[:, :])
            wo_t = []
            for h in range(H):
                t = cpool.tile([64, D], bf16, tag=f"wo{h}", name=f"wo{h}")
                nc.scalar.dma_start(out=t[:, :], in_=woT[h * 64:(h + 1) * 64, :])
                wo_t.append(t)
            # persistent per-core state
            qt = [cpool.tile([128, QC], f32r, tag=f"qt{db}", name=f"qt{db}") for db in range(NDB)]
            va = [cpool.tile([128, 8 * 65], bf16, tag=f"va{sb}", name=f"va{sb}") for sb in range(NKB)]
            mk = [cpool.tile([128, QC], bf16, tag=f"mk{kb}", name=f"mk{kb}") for kb in range(NKB)]

            # ================= PHASE A: projections =================
            with tc.tile_pool(name="apool", bufs=1) as apool:
                win = [apool.tile([128, D], f32r, tag=f"win{kc}", bufs=2, name=f"win{kc}")
                       for kc in range(NDB)]

                xq = [apool.tile([128, QC], f32r, tag=f"xq{kc}", name=f"xq{kc}") for kc in range(NDB)]
                for kc in range(NDB):
                    nc.scalar.dma_start(out=xq[kc][:, :],
                                        in_=xqT[kc * 128:(kc + 1) * 128, :])

                # ---- V projection (v natural [s, d] -> va bf16 + ones col) ----
                for kc in range(NDB):
                    nc.sync.dma_start(out=win[kc][:, :],
                                      in_=wvT[kc * 128:(kc + 1) * 128, :])
                for q4 in range(4):
                    xin = [apool.tile([128, QC], f32r, tag=f"xin{kc}", bufs=2, name=f"xinv{q4}{kc}")
                           for kc in range(NDB)]
                    for kc in range(NDB):
                        nc.sync.dma_start(
                            out=xin[kc][:, :],
                            in_=xvT[kc * 128:(kc + 1) * 128,
                                    q4 * QC:(q4 + 1) * QC])
                    for sbl in range(8):
                        sb = q4 * 8 + sbl
                        ps = psB.tile([128, QC], f32, tag="sc", bufs=3, name="vps")[:, 0:D]
                        for kc in range(NDB):
                            nc.tensor.matmul(
                                ps[:, :],
                                xin[kc][:, sbl * 128:(sbl + 1) * 128],
                                win[kc][:, :],
                                start=(kc == 0), stop=False)
                        nc.tensor.matmul(ps[:, :], ones_r[0:1, :], bv_t[0:1, :],
                                         start=False, stop=True)
                        dst = va[sb][:, :].rearrange("p (h c) -> p h c", c=65)
                        src2 = ps[:, :].rearrange("p (h c) -> p h c", c=64)
                        nc.vector.tensor_copy(dst[:, :, 0:64], src2[:, :, :])
                        nc.vector.memset(dst[:, :, 64:65], 1.0)

                # ---- Q^T projection (resident) ----
                for kc in range(NDB):
                    nc.sync.dma_start(out=win[kc][:, :],
                                      in_=wqT[kc * 128:(kc + 1) * 128, :])
                for db in range(NDB):
                    ps = psB.tile([128, QC], f32, tag="sc", bufs=3)
                    for ch in range(2):
                        for kc in range(NDB):
                            nc.tensor.matmul(
                                ps[:, ch * 512:(ch + 1) * 512],
                                win[kc][:, db * 128:(db + 1) * 128],
                                xq[kc][:, ch * 512:(ch + 1) * 512],
                                start=(kc == 0), stop=(kc == NDB - 1))
                    nc.vector.tensor_scalar_add(qt[db][:, :], ps[:, :],
                                                bq_t[:, db:db + 1])

                # ---- K^T projection -> spill to DRAM ----
                for kc in range(NDB):
                    nc.sync.dma_start(out=win[kc][:, :],
                                      in_=wkT[kc * 128:(kc + 1) * 128, :])
                for q4 in range(4):
                    g = q4
                    xin = [apool.tile([128, QC], f32r, tag=f"xin{kc}", bufs=2, name=f"xink{q4}{kc}")
                           for kc in range(NDB)]
                    for kc in range(NDB):
                        nc.sync.dma_start(
                            out=xin[kc][:, :],
                            in_=xkT[kc * 128:(kc + 1) * 128,
                                    q4 * QC:(q4 + 1) * QC])
                    for db in range(NDB):
                        ps = psB.tile([128, QC], f32, tag="sc", bufs=3)
                        for ch in range(2):
                            for kc in range(NDB):
                                nc.tensor.matmul(
                                    ps[:, ch * 512:(ch + 1) * 512],
                                    win[kc][:, db * 128:(db + 1) * 128],
                                    xin[kc][:, ch * 512:(ch + 1) * 512],
                                    start=(kc == 0), stop=(kc == NDB - 1))
                        kstg = apool.tile([128, QC], f32r, tag="kstg", bufs=3)
                        nc.vector.tensor_scalar_add(kstg[:, :], ps[:, :],
                                                    bk_t[:, db:db + 1])
                        nc.sync.dma_start(
                            out=ktdram[db, :, g * QC:(g + 1) * QC],
                            in_=kstg[:, :])

            # ---- maskT loads (host provides transposed bf16 mask) ----
            for kb in range(NKB):
                nc.scalar.dma_start(out=mk[kb][:, :],
                                    in_=masktd[kb * 128:(kb + 1) * 128, :])

            if debug_taps:
                with tc.tile_pool(name="dbgp", bufs=1) as dbgp:
                    for db in range(NDB):
                        t = dbgp.tile([128, QC], f32, tag="dq", bufs=2)
                        nc.vector.tensor_copy(t[:, :], qt[db][:, :].bitcast(f32))
                        nc.sync.dma_start(out=dbg_qt[db, :, :], in_=t[:, :])
                    for sb in range(NKB):
                        t = dbgp.tile([128, 520], f32, tag="dv", bufs=2)
                        nc.vector.tensor_copy(t[:, :], va[sb][:, :])
                        nc.sync.dma_start(out=dbg_va[sb, :, :], in_=t[:, :])
                    for kb in range(NKB):
                        t = dbgp.tile([128, QC], f32, tag="dm", bufs=2)
                        nc.vector.tensor_copy(t[:, :], mk[kb][:, :])
                        nc.sync.dma_start(out=dbg_mk[kb, :, :], in_=t[:, :])

            # ================= PHASE B: attention =================
            with tc.tile_pool(name="bpool", bufs=1) as bpool:
              at = [bpool.tile([64, QC], bf16, tag=f"at{h}", name=f"at{h}") for h in range(H)]
              LAG = 3       # PV matmul trails scores by LAG steps (hides exp+mask)
              NORM_DELAY = 3
              steps = [(hp, hh, kb) for hp in range(4) for hh in range(2)
                       for kb in range(NKB)]
              pvt = {}
              pend_pv = []      # (h, kb, probs_tile)
              pend_norm = []    # [delay_steps_left, h, pv_tile]

              def emit_pv(h, kb, probs):
                  for ch in range(2):
                      nc.tensor.matmul(
                          pvt[h][:, ch * 512:(ch + 1) * 512],
                          va[kb][:, h * 65:(h + 1) * 65],
                          probs[:, ch * 512:(ch + 1) * 512],
                          start=(kb == 0), stop=(kb == NKB - 1))

              def emit_norm(h):
                  pv = pvt[h]
                  stg = bpool.tile([64, QC], f32, tag="stg", bufs=2, name=f"stg{h}")
                  nc.vector.tensor_copy(stg[:, :], pv[0:64, :])
                  rden = bpool.tile([1, QC], f32r, tag="rden", bufs=2, name=f"rden{h}")
                  with nc.allow_low_precision(reason="rden f32r feeds tf32 bcast"):
                      nc.vector.reciprocal(rden[:, :], pv[64:65, :])
                  for ch in range(2):
                      nc.tensor.matmul(pv[0:64, ch * 512:(ch + 1) * 512],
                                       ones_r[0:1, 0:64],
                                       rden[0:1, ch * 512:(ch + 1) * 512],
                                       start=True, stop=True)
                  nc.vector.tensor_tensor(at[h][:, :], stg[:, :],
                                          pv[0:64, :], op=MULT)

              ktloc_by_hp = {}
              for hp, hh, kb in steps:
                  h = hp * 2 + hh
                  if hh == 0 and kb == 0:
                      ktloc = bpool.tile([128, S], f32r, tag="ktloc", bufs=2,
                                         name=f"ktloc{hp}")
                      nc.sync.dma_start(out=ktloc[:, :], in_=ktdram[hp, :, :])
                      ktloc_by_hp[hp] = ktloc
                  if kb == 0:
                      pvt[h] = psB.tile([65, QC], f32, tag="pv", bufs=1,
                                        name=f"pv{h}")
                  ktloc = ktloc_by_hp[hp]
                  ps = psB.tile([128, QC], f32, tag="sc", bufs=3)
                  for ch in range(2):
                      nc.tensor.matmul(
                          ps[:, ch * 512:(ch + 1) * 512],
                          ktloc[hh * 64:(hh + 1) * 64,
                                kb * 128:(kb + 1) * 128],
                          qt[hp][hh * 64:(hh + 1) * 64,
                                 ch * 512:(ch + 1) * 512],
                          start=True, stop=True)
                  probs = bpool.tile([128, QC], bf16, tag="probs", bufs=5)
                  nc.scalar.activation(probs[:, :], ps[:, :], AF.Exp)
                  nc.vector.tensor_tensor(probs[:, :], probs[:, :],
                                          mk[kb][:, :], op=MULT)
                  if debug_taps and h == 0 and kb < 2:
                      tdb = bpool.tile([128, QC], f32, tag="stg", bufs=2)
                      nc.vector.tensor_copy(tdb[:, :], probs[:, :])
                      nc.sync.dma_start(out=dbg_pr[kb, :, :], in_=tdb[:, :])
                  # advance deferred queues
                  for ent in pend_norm:
                      ent[0] -= 1
                  while pend_norm and pend_norm[0][0] <= 0:
                      emit_norm(pend_norm.pop(0)[1])
                  pend_pv.append((h, kb, probs))
                  if len(pend_pv) > LAG:
                      ph, pkb, pprobs = pend_pv.pop(0)
                      emit_pv(ph, pkb, pprobs)
                      if pkb == NKB - 1:
                          pend_norm.append([NORM_DELAY, ph])
              # drain
              while pend_pv:
                  ph, pkb, pprobs = pend_pv.pop(0)
                  emit_pv(ph, pkb, pprobs)
                  if pkb == NKB - 1:
                      pend_norm.append([NORM_DELAY, ph])
              while pend_norm:
                  emit_norm(pend_norm.pop(0)[1])
              if debug_taps:
                  for h in range(H):
                      nc.sync.dma_start(out=dbg_rd[h, :, :], in_=at[h][0:1, :])
                      tdb2 = bpool.tile([64, QC], f32, tag="stg", bufs=2)
                      nc.vector.tensor_copy(tdb2[:, :], at[h][:, :])
                      nc.sync.dma_start(out=dbg_at[h, :, :], in_=tdb2[:, :])

              # ================= PHASE C: output projection =================
              for sb in range(8):
                  po = psB.tile([128, QC], f32, tag="sc", bufs=3, name="po")[:, 0:D]
                  for h in range(H):
                      nc.tensor.matmul(
                          po[:, :],
                          at[h][:, sb * 128:(sb + 1) * 128],
                          wo_t[h][:, :],
                          start=(h == 0), stop=False)
                  nc.tensor.matmul(po[:, :], ones_r[0:1, :], bo_t[0:1, :],
                                   start=False, stop=True)
                  osb = bpool.tile([128, D], f32, tag="osb", bufs=2)
                  nc.vector.tensor_copy(osb[:, :], po[:, :])
                  nc.sync.dma_start(out=outp[sb * 128:(sb + 1) * 128, :],
                                    in_=osb[:, :])

    _legalize_waits(nc)
    return nc


_program_cache = {}
_last_in_maps = None


def _get_program():
    if "nc" not in _program_cache:
        _program_cache["nc"] = _build_program()
    return _program_cache["nc"]


def kernel(query, key, value, mask, Wq, bq, Wk, bk, Wv, bv, Wo, bo, **_unused):
    query = np.asarray(query, dtype=np.float32)
    key = np.asarray(key, dtype=np.float32)
    value = np.asarray(value, dtype=np.float32)
    mask = np.asarray(mask)

    wqT = np.ascontiguousarray(np.asarray(Wq, np.float32).T)
    wkT = np.ascontiguousarray(np.asarray(Wk, np.float32).T)
    wvT = np.ascontiguousarray(np.asarray(Wv, np.float32).T)
    woT = np.ascontiguousarray(np.asarray(Wo, np.float32).T).astype(ml_dtypes.bfloat16)
    bq_h = np.ascontiguousarray(np.asarray(bq, np.float32).reshape(NDB, 128).T)
    bk_h = np.ascontiguousarray(np.asarray(bk, np.float32).reshape(NDB, 128).T)
    bv_h = np.asarray(bv, np.float32).reshape(1, D)
    bo_h = np.asarray(bo, np.float32).reshape(1, D)

    # bf16 bits for the (0/1) mask: exact; pre-transposed per batch
    mbits = (mask != 0).astype(np.uint16) * np.uint16(0x3F80)
    mbitsT = [np.ascontiguousarray(mbits[b].T) for b in range(B)]

    xT = {}
    for b in range(B):
        xT[("q", b)] = np.ascontiguousarray(query[b].T)
        xT[("k", b)] = np.ascontiguousarray(key[b].T)
        xT[("v", b)] = np.ascontiguousarray(value[b].T)

    in_maps = []
    for c in range(NC):
        b, qs = divmod(c, 4)
        in_maps.append({
            "xqT": np.ascontiguousarray(xT[("q", b)][:, qs * QC:(qs + 1) * QC]),
            "xkT": xT[("k", b)],
            "xvT": xT[("v", b)],
            "masktd": np.ascontiguousarray(
                mbitsT[b][:, qs * QC:(qs + 1) * QC]).view(ml_dtypes.bfloat16),
            "wqT": wqT, "wkT": wkT, "wvT": wvT, "woT": woT,
            "bq_d": bq_h, "bk_d": bk_h, "bv_d": bv_h, "bo_d": bo_h,
        })

    global _last_in_maps
    _last_in_maps = in_maps
    nc = _get_program()
    res = run_bass_kernel_spmd(nc, in_maps, list(range(NC)))

    out = np.empty((B, S, D), np.float32)
    for c in range(NC):
        b, qs = divmod(c, 4)
        out[b, qs * QC:(qs + 1) * QC, :] = res.results[c]["out"]
    return out



# revision 14
# speedup vs baseline: 1.0742x; 1.0742x over previous
"""Multi-head attention Trainium2 kernel (B=2, S=4096, D=512, H=8).

Sharding: 8 cores = (batch b = c//4) x (query chunk qs = c%4 of 1024 rows).
Each core: full K/V projections for its batch, Q projection for its 1024
queries, scores/softmax/PV for all 8 heads, output projection.

v2 design (vs v1 baseline at 660us):
  - everything bf16 on the matmul paths (1 cyc/col streams, half DMA/SBUF)
  - K^T kept resident in SBUF (no DRAM spill round-trip)
  - attention processed in head PAIRS: the two heads of a d-block live at
    partitions 0:64 / 64:128, so score matmuls for the pair are row-tiled
    (tile_position (0,0)/(64,0) auto-derived) and run concurrently on the
    PE with LDWEIGHTS hidden (~536ns per pair per kb per 1024 queries).
  - mask streamed per (hp, kb) from DRAM (bufs=8 prefetch) instead of
    resident; softmax denominator via appended ones-column in va.
  - reciprocal via custom-DVE reciprocal_approx_fast (~5x faster).
  - PSUM: 4 banks scores rotation (2 x [128,1024]) + 4 banks pv pair.
  - emission order interleaves projections d-blocks 1-3 between head pairs.
"""

import numpy as np
import ml_dtypes

import bass_rust
import concourse.bass as bass
import concourse.mybir as mybir
from concourse.bass_utils import run_bass_kernel_spmd
from concourse.tile import TileContext

f32 = mybir.dt.float32
f32r = mybir.dt.float32r
bf16 = mybir.dt.bfloat16
f16 = mybir.dt.float16
AF = mybir.ActivationFunctionType
MULT = mybir.AluOpType.mult

B, S, D, H, HD = 2, 4096, 512, 8, 64
QC = 1024          # query rows per core
NC = 8             # cores
NKB = S // 128     # 32 k-blocks
NDB = D // 128     # 4 d-blocks (head pairs)

_waitfix = [0]


def _legalize_waits(nc):
    """This walrus build accepts at most one sem-wait per instruction.
    Hoist extra waits onto same-engine NOPs inserted just before."""
    for fn in nc.m.functions:
        for bb in fn.blocks:
            out, changed = [], False
            for inst in bb.instructions:
                si = inst.sync_info
                if si is not None and len(si.on_wait) > 1:
                    waits = list(si.on_wait)
                    for w in waits[:-1]:
                        _waitfix[0] += 1
                        nop = mybir.InstNoOp(
                            name=f"I-waitfix-{_waitfix[0]}", ins=[], outs=[])
                        nop.engine = inst.engine
                        nop.sync_info = bass_rust.SyncInfo(on_wait=[w], on_update=[])
                        out.append(nop)
                    inst.sync_info = bass_rust.SyncInfo(
                        on_wait=[waits[-1]], on_update=list(si.on_update))
                    changed = True
                out.append(inst)
            if changed:
                bb.instructions = out


def _build_program(with_bias=False):
    nc = bass.Bass(target_bir_lowering=False, debug=False)

    xqT = nc.dram_tensor("xqT", [D, QC], f16, kind="ExternalInput")
    xkT = nc.dram_tensor("xkT", [D, S], f16, kind="ExternalInput")
    xvT = nc.dram_tensor("xvT", [D, S], f16, kind="ExternalInput")
    masktd = nc.dram_tensor("masktd", [S, QC], bf16, kind="ExternalInput")
    wqT = nc.dram_tensor("wqT", [D, D], f16, kind="ExternalInput")
    wkT = nc.dram_tensor("wkT", [D, D], f16, kind="ExternalInput")
    wvT = nc.dram_tensor("wvT", [D, D], f16, kind="ExternalInput")
    woT = nc.dram_tensor("woT", [D, D], f16, kind="ExternalInput")
    bq_d = nc.dram_tensor("bq_d", [128, NDB], f32, kind="ExternalInput")
    bk_d = nc.dram_tensor("bk_d", [128, NDB], f32, kind="ExternalInput")
    bv_d = nc.dram_tensor("bv_d", [1, D], f16, kind="ExternalInput")
    bo_d = nc.dram_tensor("bo_d", [1, D], f16, kind="ExternalInput")
    outp = nc.dram_tensor("out", [QC, D], f32, kind="ExternalOutput")

    with TileContext(nc) as tc:
        with tc.tile_pool(name="cpool", bufs=1) as cpool, \
             tc.tile_pool(name="bpool", bufs=1) as bpool, \
             tc.tile_pool(name="psB", bufs=1, space="PSUM") as psB:
            # ---- constants / persistent ----
            ones_f = cpool.tile([1, 128], f32, tag="ones_f")
            nc.vector.memset(ones_f[:, :], 1.0)
            ones_r = cpool.tile([1, 128], f32r, tag="ones_r")
            nc.vector.tensor_copy(ones_r[:, :], ones_f[:, :])
            ones_b = cpool.tile([1, 128], f16, tag="ones_b")
            nc.vector.tensor_copy(ones_b[:, :], ones_f[:, :])
            bq_t = cpool.tile([128, NDB], f32, tag="bq")
            bk_t = cpool.tile([128, NDB], f32, tag="bk")
            bv_t = cpool.tile([1, D], f16, tag="bv")
            bo_t = cpool.tile([1, D], f16, tag="bo")
            nc.scalar.dma_start(out=bq_t[:, :], in_=bq_d[:, :])
            nc.scalar.dma_start(out=bk_t[:, :], in_=bk_d[:, :])
            nc.scalar.dma_start(out=bv_t[:, :], in_=bv_d[:, :])
            nc.scalar.dma_start(out=bo_t[:, :], in_=bo_d[:, :])
            wo_t = []
            for h in range(H):
                t = cpool.tile([64, D], f16, tag=f"wo{h}", name=f"wo{h}")
                nc.scalar.dma_start(out=t[:, :], in_=woT[h * 64:(h + 1) * 64, :])
                wo_t.append(t)
            wq_c, wk_c, wv_c = [], [], []
            for kc in range(NDB):
                t = cpool.tile([128, D], f16, tag=f"wq{kc}", name=f"wq{kc}")
                nc.scalar.dma_start(out=t[:, :], in_=wqT[kc * 128:(kc + 1) * 128, :])
                wq_c.append(t)
                t = cpool.tile([128, D], f16, tag=f"wk{kc}", name=f"wk{kc}")
                nc.scalar.dma_start(out=t[:, :], in_=wkT[kc * 128:(kc + 1) * 128, :])
                wk_c.append(t)
                t = cpool.tile([128, D], f16, tag=f"wv{kc}", name=f"wv{kc}")
                nc.scalar.dma_start(out=t[:, :], in_=wvT[kc * 128:(kc + 1) * 128, :])
                wv_c.append(t)
            xq_c, xk_c = [], []
            for kc in range(NDB):
                t = cpool.tile([128, QC], f16, tag=f"xq{kc}", name=f"xq{kc}")
                nc.sync.dma_start(out=t[:, :], in_=xqT[kc * 128:(kc + 1) * 128, :])
                xq_c.append(t)
                t = cpool.tile([128, S], f16, tag=f"xk{kc}", name=f"xk{kc}")
                nc.sync.dma_start(out=t[:, :], in_=xkT[kc * 128:(kc + 1) * 128, :])
                xk_c.append(t)

            qt = [cpool.tile([128, QC], f16, tag=f"qt{db}", name=f"qt{db}")
                  for db in range(NDB)]
            kt = [cpool.tile([128, S], f16, tag=f"kt{db}", name=f"kt{db}")
                  for db in range(NDB)]
            va = [cpool.tile([128, 8 * 65], bf16, tag=f"va{sb}", name=f"va{sb}")
                  for sb in range(NKB)]
            at = [cpool.tile([64, QC], f16, tag=f"at{h}", name=f"at{h}")
                  for h in range(H)]

            # ---------- projection emitters ----------
            def emit_k_chunk(db, g):
                """kt[db][:, g*1024:(g+1)*1024] from xk chunk g."""
                ps = psB.tile([128, QC], f32, tag="sc", bufs=2, name=f"kps{db}{g}")
                for ch in range(2):
                    for kc in range(NDB):
                        nc.tensor.matmul(
                            ps[:, ch * 512:(ch + 1) * 512],
                            wk_c[kc][:, db * 128:(db + 1) * 128],
                            xk_c[kc][:, g * QC + ch * 512:g * QC + (ch + 1) * 512],
                            start=(kc == 0), stop=(kc == NDB - 1))
                nc.scalar.activation(kt[db][:, g * QC:(g + 1) * QC], ps[:, :],
                                     AF.Identity, bias=bk_t[:, db:db + 1])

            def emit_q_chunk(db):
                ps = psB.tile([128, QC], f32, tag="sc", bufs=2, name=f"qps{db}")
                for ch in range(2):
                    for kc in range(NDB):
                        nc.tensor.matmul(
                            ps[:, ch * 512:(ch + 1) * 512],
                            wq_c[kc][:, db * 128:(db + 1) * 128],
                            xq_c[kc][:, ch * 512:(ch + 1) * 512],
                            start=(kc == 0), stop=(kc == NDB - 1))
                nc.scalar.activation(qt[db][:, :], ps[:, :],
                                     AF.Identity, bias=bq_t[:, db:db + 1])

            def emit_v_chunk(sb):
                xv_t = [bpool.tile([128, 128], f16, tag=f"xv{kc}", bufs=2,
                                   name=f"xv{sb}{kc}") for kc in range(NDB)]
                for kc in range(NDB):
                    nc.sync.dma_start(
                        out=xv_t[kc][:, :],
                        in_=xvT[kc * 128:(kc + 1) * 128, sb * 128:(sb + 1) * 128])
                ps = psB.tile([128, QC], f32, tag="sc", bufs=2,
                              name=f"vps{sb}")[:, 0:D]
                for kc in range(NDB):
                    nc.tensor.matmul(ps[:, :], xv_t[kc][:, :], wv_c[kc][:, :],
                                     start=(kc == 0),
                                     stop=(kc == NDB - 1 and not with_bias))
                if with_bias:
                    nc.tensor.matmul(ps[:, :], ones_b[0:1, :], bv_t[0:1, :],
                                     start=False, stop=True)
                dst = va[sb][:, :].rearrange("p (h c) -> p h c", c=65)
                src = ps[:, :].rearrange("p (h c) -> p h c", c=64)
                with nc.allow_low_precision(reason="bf16 va"):
                    nc.vector.tensor_copy(dst[:, :, 0:64], src[:, :, :])
                nc.vector.memset(dst[:, :, 64:65], 1.0)

            # ---------- pre-attention: d-block 0 + all of V ----------
            emit_q_chunk(0)
            for g in range(4):
                emit_k_chunk(0, g)
            for sb in range(NKB):
                emit_v_chunk(sb)

            # ---------- attention in head pairs ----------
            LAG = 2            # pair-steps PV trails behind scores
            NORM_DELAY = 2     # pair-steps before norm after last PV
            pvt = {}
            pend_pv = []       # (h, kb, probs)
            pend_norm = []     # [delay, h]

            def emit_pv(h, kb, probs):
                for ch in range(2):
                    nc.tensor.matmul(
                        pvt[h][:, ch * 512:(ch + 1) * 512],
                        va[kb][:, h * 65:(h + 1) * 65],
                        probs[:, ch * 512:(ch + 1) * 512],
                        start=(kb == 0), stop=(kb == NKB - 1))

            def emit_norm(h):
                pv = pvt[h]
                stg = bpool.tile([64, QC], f32, tag="stg", bufs=1,
                                 name=f"stg{h}")
                nc.vector.tensor_copy(stg[:, :], pv[0:64, :])
                rden_r = bpool.tile([1, QC], f32r, tag="rden", bufs=1,
                                    name=f"rden{h}")
                with nc.allow_low_precision(reason="rden f32r feeds bcast mm"):
                    nc.vector.reciprocal(rden_r[:, :], pv[64:65, :])
                bc = psB.tile([128, QC], f32, tag="sc", bufs=2,
                              name=f"bc{h}")
                for ch in range(2):
                    nc.tensor.matmul(bc[0:64, ch * 512:(ch + 1) * 512],
                                     ones_r[0:1, 0:64],
                                     rden_r[0:1, ch * 512:(ch + 1) * 512],
                                     start=True, stop=True)
                with nc.allow_low_precision(reason="bf16 at"):
                    nc.vector.tensor_tensor(at[h][:, :], stg[:, :],
                                            bc[0:64, :], op=MULT)

            def b_step(hp, kb):
                h_e, h_o = 2 * hp, 2 * hp + 1
                mk_t = bpool.tile([128, QC], bf16, tag="mk", bufs=6,
                                  name=f"mk{hp}_{kb}")
                nc.gpsimd.dma_start(out=mk_t[:, :],
                                    in_=masktd[kb * 128:(kb + 1) * 128, :])
                if kb == 0:
                    pvt[h_e] = psB.tile([65, QC], f32, tag="pv", bufs=2,
                                        name=f"pv{h_e}")
                    pvt[h_o] = psB.tile([65, QC], f32, tag="pv", bufs=2,
                                        name=f"pv{h_o}")
                psE = psB.tile([128, QC], f32, tag="sc", bufs=2,
                               name=f"sE{hp}_{kb}")
                psO = psB.tile([128, QC], f32, tag="sc", bufs=2,
                               name=f"sO{hp}_{kb}")
                kbs = slice(kb * 128, (kb + 1) * 128)
                for ch in range(2):
                    chs = slice(ch * 512, (ch + 1) * 512)
                    nc.tensor.matmul(psE[:, chs], kt[hp][0:64, kbs],
                                     qt[hp][0:64, chs], start=True, stop=True)
                    nc.tensor.matmul(psO[:, chs], kt[hp][64:128, kbs],
                                     qt[hp][64:128, chs], start=True, stop=True)
                probs_e = bpool.tile([128, QC], bf16, tag="probs", bufs=5,
                                     name=f"pe{hp}_{kb}")
                nc.scalar.activation(probs_e[:, :], psE[:, :], AF.Exp)
                nc.vector.tensor_tensor(probs_e[:, :], probs_e[:, :],
                                        mk_t[:, :], op=MULT)
                probs_o = bpool.tile([128, QC], bf16, tag="probs", bufs=5,
                                     name=f"po{hp}_{kb}")
                nc.scalar.activation(probs_o[:, :], psO[:, :], AF.Exp)
                nc.vector.tensor_tensor(probs_o[:, :], probs_o[:, :],
                                        mk_t[:, :], op=MULT)
                # advance deferred queues
                for ent in pend_norm:
                    ent[0] -= 1
                while pend_norm and pend_norm[0][0] <= 0:
                    emit_norm(pend_norm.pop(0)[1])
                pend_pv.append((h_e, kb, probs_e))
                pend_pv.append((h_o, kb, probs_o))
                while len(pend_pv) > 2 * LAG:
                    ph, pkb, pprobs = pend_pv.pop(0)
                    emit_pv(ph, pkb, pprobs)
                    if pkb == NKB - 1:
                        pend_norm.append([NORM_DELAY, ph])

            for hp in range(NDB):
                for kb in range(NKB):
                    b_step(hp, kb)
                # projections for the next head pair between pairs
                if hp + 1 < NDB:
                    emit_q_chunk(hp + 1)
                    for g in range(4):
                        emit_k_chunk(hp + 1, g)
            # drain
            while pend_pv:
                ph, pkb, pprobs = pend_pv.pop(0)
                emit_pv(ph, pkb, pprobs)
                if pkb == NKB - 1:
                    pend_norm.append([NORM_DELAY, ph])
            while pend_norm:
                emit_norm(pend_norm.pop(0)[1])

            # ---------- output projection ----------
            for sb in range(8):
                po = psB.tile([128, QC], f32, tag="sc", bufs=2,
                              name=f"po{sb}")[:, 0:D]
                for h in range(H):
                    nc.tensor.matmul(
                        po[:, :],
                        at[h][:, sb * 128:(sb + 1) * 128],
                        wo_t[h][:, :],
                        start=(h == 0), stop=(h == H - 1 and not with_bias))
                if with_bias:
                    nc.tensor.matmul(po[:, :], ones_b[0:1, :], bo_t[0:1, :],
                                     start=False, stop=True)
                osb = bpool.tile([128, D], f32, tag="osb", bufs=2,
                                 name=f"osb{sb}")
                nc.vector.tensor_copy(osb[:, :], po[:, :])
                nc.sync.dma_start(out=outp[sb * 128:(sb + 1) * 128, :],
                                  in_=osb[:, :])

    _legalize_waits(nc)
    return nc


_program_cache = {}
_last_in_maps = None


def _get_program(with_bias=False):
    key = ("nc", with_bias)
    if key not in _program_cache:
        _program_cache[key] = _build_program(with_bias)
    return _program_cache[key]


def kernel(query, key, value, mask, Wq, bq, Wk, bk, Wv, bv, Wo, bo, **_unused):
    query = np.asarray(query, dtype=np.float32)
    key = np.asarray(key, dtype=np.float32)
    value = np.asarray(value, dtype=np.float32)
    mask = np.asarray(mask)

    bq_h = np.ascontiguousarray(np.asarray(bq, np.float32).reshape(NDB, 128).T)
    bk_h = np.ascontiguousarray(np.asarray(bk, np.float32).reshape(NDB, 128).T)
    bv_h = np.asarray(bv, np.float32).reshape(1, D).astype(np.float16)
    bo_h = np.asarray(bo, np.float32).reshape(1, D).astype(np.float16)
    with_bias = bool(np.any(np.asarray(bq)) or np.any(np.asarray(bk))
                     or np.any(np.asarray(bv)) or np.any(np.asarray(bo)))

    wqT = np.ascontiguousarray(np.asarray(Wq, np.float32).T).astype(np.float16)
    wkT = np.ascontiguousarray(np.asarray(Wk, np.float32).T).astype(np.float16)
    wvT = np.ascontiguousarray(np.asarray(Wv, np.float32).T).astype(np.float16)
    woT = np.ascontiguousarray(np.asarray(Wo, np.float32).T).astype(np.float16)

    # bf16 bits for the (0/1) mask: exact; pre-transposed per batch
    mbits = (mask != 0).astype(np.uint16) * np.uint16(0x3F80)
    mbitsT = [np.ascontiguousarray(mbits[b].T) for b in range(B)]

    xT = {}
    for b in range(B):
        xT[("q", b)] = np.ascontiguousarray(query[b].T).astype(np.float16)
        xT[("k", b)] = np.ascontiguousarray(key[b].T).astype(np.float16)
        xT[("v", b)] = np.ascontiguousarray(value[b].T).astype(np.float16)

    in_maps = []
    for c in range(NC):
        b, qs = divmod(c, 4)
        in_maps.append({
            "xqT": np.ascontiguousarray(xT[("q", b)][:, qs * QC:(qs + 1) * QC]),
            "xkT": xT[("k", b)],
            "xvT": xT[("v", b)],
            "masktd": np.ascontiguousarray(
                mbitsT[b][:, qs * QC:(qs + 1) * QC]).view(ml_dtypes.bfloat16),
            "wqT": wqT, "wkT": wkT, "wvT": wvT, "woT": woT,
            "bq_d": bq_h, "bk_d": bk_h, "bv_d": bv_h, "bo_d": bo_h,
        })

    global _last_in_maps
    _last_in_maps = in_maps
    nc = _get_program(with_bias)
    res = run_bass_kernel_spmd(nc, in_maps, list(range(NC)))

    out = np.empty((B, S, D), np.float32)
    for c in range(NC):
        b, qs = divmod(c, 4)
        out[b, qs * QC:(qs + 1) * QC, :] = res.results[c]["out"]
    return out
